# revision 37
# baseline (speedup 1.0000x reference)
"""Trainium2 Bass kernel for a 2-layer GCN + linear head (SPMD over 8 cores).

Strategy: nodes sharded across 8 cores; edges partitioned by target node.
GCN normalization (deg/dinv) is folded into edge values host-side:
S values = w_e * dinv[src] * dinv[tgt]; the self-loop becomes a per-window
matmul against diag(dinv^2). Per layer, per core:
  1. table tiles xw = h @ W ([row, feat], bf16) -> DRAM shard, AllGather to a
     full padded table ([8*nwpad, 128]) in each core's DRAM,
  2. per (window-group g of 8, source-chunk c of 2 shards): one dma_gather of
     g*tpb tiles (up to 5120 rows) from chunk c,
  3. S tiles built on DVE in transposed (t-major, s-minor) layout so every
     operand has a packed last dim (2 elem/cycle fast path),
  4. scatter matmuls out[feat, tgt] += gathered.T @ S accumulate in PSUM over
     chunks; + self-loop matmul lhsT=xw_own, rhs=diag(dinv2);
  5. retire: h = Relu(psum + b) via one Activation op -> [feat, tgt] tiles
     which are directly the lhsT for the next layer's table matmul.
Head: out[tgt, 16] = h2.T @ Wout.T + bout per window, one batched DMA out.
"""

import os
import sys

sys.path.insert(0, "/opt/trn_rl_repo")

SKIP_MAIN = os.environ.get("KERNEL_SKIP_MAIN") == "1"
SKIP_AG = os.environ.get("KERNEL_SKIP_AG") == "1"
REPS = int(os.environ.get("KERNEL_REPS", "1"))
CALL_TILES = int(os.environ.get("KERNEL_CALL_TILES", "8"))  # <=8 proven on HW
SCRATCH = int(os.environ.get("KERNEL_SCRATCH", "16384"))
FP8 = os.environ.get("KERNEL_FP8") == "1"  # fp8e4m3 gather table
GATHER_ONLY = os.environ.get("KERNEL_GATHER_ONLY") == "1"
QUEUES = int(os.environ.get("KERNEL_QUEUES", "4"))
TRIM = os.environ.get("KERNEL_TRIM", "1") == "1"  # -1 pad + count registers
TRIM_FULL = os.environ.get("KERNEL_TRIM_FULL") == "1"  # debug: reg=full, pad=0

import numpy as np
import ml_dtypes

import concourse.bass as bass
import concourse.mybir as mybir
import concourse.tile as tile
from concourse import bacc, library_config
from concourse.bass_utils import run_bass_kernel_spmd

BF16 = mybir.dt.bfloat16
F32 = mybir.dt.float32
I16 = mybir.dt.int16

NCORES = 8
F = 128
NPRED = 16
G = 8  # windows per group (PSUM: G tags x 2 bufs = 16 x 512B/part)


class Cfg:
    def __init__(self, n_nodes, cap):
        self.n = n_nodes
        self.per = n_nodes // NCORES
        self.nwin = (self.per + 127) // 128
        self.nwpad = self.nwin * 128
        # 4 window-aligned quarters of each core's padded shard; chunk c of
        # the table = all cores' quarter c (one AllGather per quarter)
        self.nchunk = 4
        qw = [(self.nwin + 3 - q) // 4 for q in range(4)]  # windows/quarter
        self.chunk_rows = [w * 128 for w in qw]  # local rows per quarter
        self.chunk_lo = np.concatenate([[0], np.cumsum(self.chunk_rows)]).astype(int)
        self.chspan = [NCORES * r for r in self.chunk_rows]
        assert max(self.chspan) <= 32768
        self.chunk_base = np.concatenate([[0], np.cumsum(self.chspan)]).astype(int)
        self.ntab = NCORES * self.nwpad
        self.cap = cap
        self.tpb = cap // 128
        self.groups = []
        w = 0
        while w < self.nwin:
            g = min(G, self.nwin - w)
            self.groups.append(list(range(w, w + g)))
            w += g
        # slot layout: for g: for c: for w in g: cap  (one gather call per (g,c))
        self.bucket_base = {}
        pos = 0
        for wl in self.groups:
            for c in range(self.nchunk):
                for w in wl:
                    self.bucket_base[(w, c)] = pos
                    pos += cap
        self.nslot = pos
        self.ntile = pos // 128
        # distinct group tile-counts (gt) for iotaT constants
        self.gts = sorted({len(wl) * self.tpb for wl in self.groups})
        self.iota_off = {}
        off = 0
        for gt in self.gts:
            self.iota_off[gt] = off
            off += gt * 128
        self.iota_len = off
        self.max_gt = max(self.gts)


def build_nc(cfg: Cfg):
    nc = bacc.Bacc(
        "TRN2",
        target_bir_lowering=False,
        num_swdge_queues=QUEUES,
        dynamic_dma_scratch_size=SCRATCH,
    )
    nwin, nwpad, tpb = cfg.nwin, cfg.nwpad, cfg.tpb

    # per-core external inputs
    xTloc = nc.dram_tensor("xTloc", [128, nwpad], BF16, kind="ExternalInput")
    idx_d = nc.dram_tensor("idx", [128, cfg.nslot // 16], I16, kind="ExternalInput")
    colv_d = nc.dram_tensor("colv", [128, cfg.ntile], BF16, kind="ExternalInput")
    wv_d = nc.dram_tensor("wv", [128, cfg.ntile], BF16, kind="ExternalInput")
    w1_d = nc.dram_tensor("w1", [128, 128], BF16, kind="ExternalInput")
    w2_d = nc.dram_tensor("w2", [128, 128], BF16, kind="ExternalInput")
    wout_d = nc.dram_tensor("woutT", [128, NPRED], BF16, kind="ExternalInput")
    b1_d = nc.dram_tensor("b1", [128, 1], F32, kind="ExternalInput")
    b2_d = nc.dram_tensor("b2", [128, 1], F32, kind="ExternalInput")
    bout_d = nc.dram_tensor("boutbc", [128, NPRED], F32, kind="ExternalInput")
    dinv2_d = nc.dram_tensor("dinv2", [128, nwin], F32, kind="ExternalInput")
    counts_d = nc.dram_tensor(
        "counts", [1, nwin * cfg.nchunk], mybir.dt.int32, kind="ExternalInput"
    )
    iota_d = nc.dram_tensor("iotaT", [128, cfg.iota_len], BF16, kind="ExternalInput")
    ident_d = nc.dram_tensor("ident", [128, 128], BF16, kind="ExternalInput")

    outT = nc.dram_tensor("outT2", [nwpad, NPRED], F32, kind="ExternalOutput")

    # internal DRAM; tables Shared so the AllGather writes one buffer per
    # chip instead of a full copy per core
    SHARED = os.environ.get("KERNEL_NO_SHARED") != "1"
    tkw = {"addr_space": "Shared"} if SHARED else {}
    table1 = nc.dram_tensor("table1", [cfg.ntab, 128], BF16, **tkw)
    table2 = nc.dram_tensor("table2", [cfg.ntab, 128], BF16, **tkw)
    ag_in = nc.dram_tensor("ag_in", [nwpad, 128], BF16)

    qctr = [0]

    with tile.TileContext(nc) as tc:
        with (
            tc.tile_pool(name="const", bufs=1) as cp,
            tc.tile_pool(name="big", bufs=3) as bigp,
            tc.tile_pool(name="gat", bufs=3) as gp,
            tc.tile_pool(name="sbuild", bufs=3) as sp_,
            tc.tile_pool(name="idxp", bufs=4) as idxp,
            tc.tile_pool(name="dp", bufs=4) as dp,
        ):
            nc.gpsimd.load_library(library_config.mlp)
            iota_t = cp.tile([128, cfg.iota_len], BF16)
            nc.sync.dma_start(iota_t[:], iota_d[:])
            ident_t = cp.tile([128, 128], BF16)
            nc.sync.dma_start(ident_t[:], ident_d[:])
            w1_t = cp.tile([128, 128], BF16)
            nc.sync.dma_start(w1_t[:], w1_d[:])
            w2_t = cp.tile([128, 128], BF16)
            nc.sync.dma_start(w2_t[:], w2_d[:])
            wout_t = cp.tile([128, NPRED], BF16)
            nc.sync.dma_start(wout_t[:], wout_d[:])
            b1_t = cp.tile([128, 1], F32)
            nc.sync.dma_start(b1_t[:], b1_d[:])
            b2_t = cp.tile([128, 1], F32)
            nc.sync.dma_start(b2_t[:], b2_d[:])
            bout_t = cp.tile([128, NPRED], F32)
            nc.sync.dma_start(bout_t[:], bout_d[:])
            dinv2_t = cp.tile([128, nwin], F32)
            nc.sync.dma_start(dinv2_t[:], dinv2_d[:])
            colv_t = cp.tile([128, cfg.ntile], BF16)
            nc.sync.dma_start(colv_t[:], colv_d[:])
            wv_t = cp.tile([128, cfg.ntile], BF16)
            nc.sync.dma_start(wv_t[:], wv_d[:])
            xTl_t = cp.tile([128, nwpad], BF16)
            nc.sync.dma_start(xTl_t[:], xTloc[:])
            counts_t = cp.tile([1, nwin * cfg.nchunk], mybir.dt.int32)
            nc.sync.dma_start(counts_t[:], counts_d[:])
            out_sb = cp.tile([128, nwin * NPRED], F32)

            import itertools

            _ctr = itertools.count()

            def build_table(lhsT_full, w_t, table):
                """xw_own[:, w*128:] = (lhsT_full.T @ w_t) per window ([row, feat]
                bf16); one batched DMA to ag_in; AllGather -> table."""
                xw_own = bigp.tile([128, nwpad], BF16, tag="big", name="xw_own")
                with tc.tile_pool(name=f"pst{next(_ctr)}", bufs=3, space="PSUM") as pb:
                    for w in range(nwin):
                        cs = slice(w * 128, (w + 1) * 128)
                        ps = pb.tile([128, 128], F32, tag="pb")
                        nc.tensor.matmul(
                            ps[:], lhsT=lhsT_full[:, cs], rhs=w_t[:], start=True, stop=True
                        )
                        nc.scalar.copy(out=xw_own[:, cs], in_=ps[:])
                # per-quarter shard write + AllGather so chunk-c gathers can
                # start as soon as AG_c lands
                for q in range(cfg.nchunk):
                    lo, hi = int(cfg.chunk_lo[q]), int(cfg.chunk_lo[q + 1])
                    nc.sync.dma_start(
                        ag_in[lo:hi, :].rearrange("(w t) f -> t w f", t=128),
                        xw_own[:, lo:hi].rearrange("p (w f) -> p w f", f=128),
                    )
                if not SKIP_AG:
                    for q in range(cfg.nchunk):
                        lo, hi = int(cfg.chunk_lo[q]), int(cfg.chunk_lo[q + 1])
                        nc.gpsimd.collective_compute(
                            "AllGather",
                            mybir.AluOpType.bypass,
                            replica_groups=[list(range(NCORES))],
                            ins=[ag_in[lo:hi, :]],
                            outs=[
                                table[
                                    int(cfg.chunk_base[q]) : int(cfg.chunk_base[q + 1]),
                                    :,
                                ]
                            ],
                        )
                return xw_own

            def main_pass(table, xw_own, b_t, rep):
                """gather + scatter-matmul + self-loop + retire -> h [feat, tgt]."""
                h = bigp.tile([128, nwpad], BF16, tag="big", name="h")
                if GATHER_ONLY:
                    nc.vector.memset(h[:], 0.0)
                with tc.tile_pool(name=f"psm{next(_ctr)}", bufs=2, space="PSUM") as pm:
                    for wl in cfg.groups:
                        gw = len(wl)
                        gt = gw * tpb
                        s0 = cfg.bucket_base[(wl[0], 0)]  # call slot base, chunk 0
                        idxt = idxp.tile(
                            [128, cfg.nchunk * gt * 8], I16, tag="idx", name="idxt"
                        )
                        nc.sync.dma_start(
                            idxt[:],
                            idx_d[:, s0 // 16 : s0 // 16 + cfg.nchunk * gt * 8],
                        )
                        gts = {}
                        for c in range(cfg.nchunk):
                            gtl = gp.tile([128, cfg.max_gt, 128], BF16, tag="g", name="g")
                            if TRIM:
                                nc.vector.memset(gtl[:, :gt, :], 0.0)
                                # one call per bucket: padding idxs are trailing
                                # -1, count register trims the descriptors
                                for wi, w in enumerate(wl):
                                    off = wi * tpb
                                    nidx = tpb * 128
                                    b = w * cfg.nchunk + c
                                    cnt = nc.gpsimd.value_load(
                                        counts_t[0:1, b : b + 1],
                                        min_val=1,
                                        max_val=nidx,
                                    )
                                    nc.gpsimd.dma_gather(
                                        gtl[:, off : off + tpb, :],
                                        table[
                                            int(cfg.chunk_base[c]) : int(
                                                cfg.chunk_base[c + 1]
                                            ),
                                            :,
                                        ],
                                        idxt[
                                            :,
                                            c * gt * 8
                                            + off * 8 : c * gt * 8
                                            + (off + tpb) * 8,
                                        ],
                                        nidx,
                                        cnt,
                                        128,
                                        queue_num=qctr[0] % QUEUES,
                                    )
                                    qctr[0] += 1
                            else:
                                off = 0
                                while off < gt:
                                    ct = min(CALL_TILES, gt - off)
                                    nidx = ct * 128
                                    nc.gpsimd.dma_gather(
                                        gtl[:, off : off + ct, :],
                                        table[
                                            int(cfg.chunk_base[c]) : int(
                                                cfg.chunk_base[c + 1]
                                            ),
                                            :,
                                        ],
                                        idxt[
                                            :,
                                            c * gt * 8
                                            + off * 8 : c * gt * 8
                                            + (off + ct) * 8,
                                        ],
                                        nidx,
                                        nidx,
                                        128,
                                        queue_num=qctr[0] % QUEUES,
                                    )
                                    qctr[0] += 1
                                    off += ct
                            gts[c] = gtl
                        if GATHER_ONLY:
                            continue
                        # pack 4 windows per PSUM bank ([128, 512] f32)
                        nbank = (gw + 3) // 4
                        banks = [
                            pm.tile([128, 512], F32, tag=f"m{i}", name=f"m{i}")
                            for i in range(nbank)
                        ]
                        ps = {
                            w: banks[i // 4][:, (i % 4) * 128 : (i % 4 + 1) * 128]
                            for i, w in enumerate(wl)
                        }
                        # S tiles, t-major: S[p, t*gt + s] for the 4 chunks
                        sbt = {}
                        for c in range(cfg.nchunk):
                            t0 = (s0 + c * gt * 128) // 128
                            sb = sp_.tile([128, cfg.max_gt * 128], BF16, tag="sb", name="sb")
                            v3 = sb[:, : gt * 128].rearrange("p (t s) -> p t s", s=gt)
                            io3 = iota_t[
                                :, cfg.iota_off[gt] : cfg.iota_off[gt] + gt * 128
                            ].rearrange("p (t s) -> p t s", s=gt)
                            cb = (
                                colv_t[:, t0 : t0 + gt]
                                .rearrange("p (x s) -> p x s", x=1)
                                .to_broadcast([128, 128, gt])
                            )
                            wb = (
                                wv_t[:, t0 : t0 + gt]
                                .rearrange("p (x s) -> p x s", x=1)
                                .to_broadcast([128, 128, gt])
                            )
                            nc.vector.tensor_tensor(
                                out=v3, in0=io3, in1=cb, op=mybir.AluOpType.is_equal
                            )
                            nc.vector.tensor_tensor(
                                out=v3, in0=v3, in1=wb, op=mybir.AluOpType.mult
                            )
                            sbt[c] = sb
                        for c in range(cfg.nchunk):
                            gtl = gts[c]
                            v3 = sbt[c][:, : gt * 128].rearrange(
                                "p (t s) -> p t s", s=gt
                            )
                            for s in range(gt):
                                w = wl[s // tpb]
                                # start=True zeroes the whole 2KB bank region;
                                # only the first matmul into each bank carries it
                                nc.tensor.matmul(
                                    ps[w][:],
                                    lhsT=gtl[:, s, :],
                                    rhs=v3[:, :, s],
                                    start=(c == 0 and s % (4 * tpb) == 0),
                                    stop=False,
                                )
                        # self-loops close each bank's group, then retires read it
                        for i, w in enumerate(wl):
                            cs = slice(w * 128, (w + 1) * 128)
                            d_t = dp.tile([128, 128], BF16, tag="d", name="d_t")
                            nc.vector.tensor_scalar(
                                out=d_t[:],
                                in0=ident_t[:],
                                scalar1=dinv2_t[:, w : w + 1],
                                scalar2=None,
                                op0=mybir.AluOpType.mult,
                            )
                            nc.tensor.matmul(
                                ps[w][:],
                                lhsT=xw_own[:, cs],
                                rhs=d_t[:],
                                start=False,
                                stop=(i % 4 == 3 or i == gw - 1),
                            )
                        for w in wl:
                            cs = slice(w * 128, (w + 1) * 128)
                            nc.scalar.activation(
                                out=h[:, cs],
                                in_=ps[w][:],
                                func=mybir.ActivationFunctionType.Relu,
                                bias=b_t[:, 0:1],
                            )
                return h

            for rep in range(REPS):
                xw1 = build_table(xTl_t, w1_t, table1)
                if SKIP_MAIN:
                    h1 = bigp.tile([128, nwpad], BF16, tag="big", name="h")
                    nc.vector.memset(h1[:], 0.0)
                else:
                    h1 = main_pass(table1, xw1, b1_t, rep)
                xw2 = build_table(h1, w2_t, table2)
                if SKIP_MAIN:
                    h2 = bigp.tile([128, nwpad], BF16, tag="big", name="h")
                    nc.vector.memset(h2[:], 0.0)
                else:
                    h2 = main_pass(table2, xw2, b2_t, rep)
                # head: out_sb[t, w*16+j] = sum_f h2[f, w*128+t] * woutT[f, j] + bout
                with tc.tile_pool(name=f"psh{next(_ctr)}", bufs=3, space="PSUM") as ph:
                    for w in range(nwin):
                        cs = slice(w * 128, (w + 1) * 128)
                        pso = ph.tile([128, NPRED], F32, tag="h")
                        nc.tensor.matmul(
                            pso[:], lhsT=h2[:, cs], rhs=wout_t[:], start=True, stop=True
                        )
                        nc.vector.tensor_tensor(
                            out=out_sb[:, w * NPRED : (w + 1) * NPRED],
                            in0=pso[:],
                            in1=bout_t[:],
                            op=mybir.AluOpType.add,
                        )
                nc.sync.dma_start(
                    outT.rearrange("(w t) j -> t w j", t=128),
                    out_sb[:].rearrange("p (w j) -> p w j", j=NPRED),
                )
    nc.compile()
    return nc


def prep_inputs(cfg: Cfg, x, edge_index, edge_weight, W1, b1, W2, b2, Wout, bout):
    per, nwin, nwpad, cap = cfg.per, cfg.nwin, cfg.nwpad, cfg.cap
    n = cfg.n
    row = np.asarray(edge_index[0], dtype=np.int64)
    col = np.asarray(edge_index[1], dtype=np.int64)
    wgt = np.asarray(edge_weight, dtype=np.float32)

    # host-side GCN normalization
    deg = (np.bincount(col, weights=wgt.astype(np.float64), minlength=n) + 1.0).astype(
        np.float32
    )
    dinv = (1.0 / np.sqrt(deg)).astype(np.float32)
    nv = dinv[row] * wgt * dinv[col]  # folded edge values

    # table positions for sources: chunk q = quarter of the padded shard,
    # table row = chunk_base[q] + core*chunk_rows[q] + (lpad - chunk_lo[q])
    src_core = row // per
    lpad = row - src_core * per
    chunk = np.searchsorted(cfg.chunk_lo[1:], lpad, side="right")
    idx16 = (
        src_core * np.asarray(cfg.chunk_rows)[chunk] + (lpad - cfg.chunk_lo[chunk])
    ).astype(np.int16)

    core = col // per
    col_local = col - core * per
    win = col_local >> 7
    cw = (col_local & 127).astype(np.float32)

    nbuck_core = nwin * cfg.nchunk
    bid = (core * nbuck_core + win * cfg.nchunk + chunk).astype(np.int64)
    order = np.argsort(bid, kind="stable")
    bid_s = bid[order]
    counts = np.bincount(bid_s, minlength=NCORES * nbuck_core)
    assert counts.max() <= cap, f"bucket overflow: {counts.max()} > {cap}"
    starts = np.zeros(NCORES * nbuck_core + 1, dtype=np.int64)
    np.cumsum(counts, out=starts[1:])
    rank = np.arange(len(order)) - starts[bid_s]

    base_1core = np.zeros(nbuck_core, dtype=np.int64)
    for (w, c), b in cfg.bucket_base.items():
        base_1core[w * cfg.nchunk + c] = b
    slot = base_1core[bid_s % nbuck_core] + rank
    core_s = bid_s // nbuck_core

    idx_all = np.full(
        (NCORES, cfg.nslot), -1 if (TRIM and not TRIM_FULL) else 0, dtype=np.int16
    )
    colv = np.zeros((NCORES, cfg.nslot), dtype=np.float32)
    wv = np.zeros((NCORES, cfg.nslot), dtype=np.float32)
    idx_all[core_s, slot] = idx16[order]
    colv[core_s, slot] = cw[order]
    wv[core_s, slot] = nv[order]

    # per-bucket true counts (>=1: an empty bucket keeps one idx-0 descriptor)
    counts2 = counts.reshape(NCORES, nwin * cfg.nchunk).astype(np.int32)
    empty = counts2 == 0
    if empty.any():
        ec, eb = np.nonzero(empty)
        for d, b in zip(ec, eb):
            w, c = b // cfg.nchunk, b % cfg.nchunk
            idx_all[d, cfg.bucket_base[(w, c)]] = 0
    counts2 = np.maximum(counts2, 1)
    if TRIM_FULL:
        counts2[:] = cfg.tpb * 128

    # 16-wrap the whole idx array (position-uniform transform, so any
    # 16-aligned sub-call slice reads its own slots)
    blk = np.transpose(idx_all.reshape(NCORES, cfg.nslot // 16, 16), (0, 2, 1))
    idx_wrapped = np.tile(blk, (1, 8, 1))

    colv_t = np.transpose(colv.reshape(NCORES, cfg.ntile, 128), (0, 2, 1)).astype(
        ml_dtypes.bfloat16
    )
    wv_t = np.transpose(wv.reshape(NCORES, cfg.ntile, 128), (0, 2, 1)).astype(
        ml_dtypes.bfloat16
    )

    xpad = np.zeros((NCORES, nwpad, 128), dtype=np.float32)
    xv = np.asarray(x, dtype=np.float32)
    for d in range(NCORES):
        xpad[d, :per] = xv[d * per : (d + 1) * per]
    xTloc = np.ascontiguousarray(np.transpose(xpad, (0, 2, 1))).astype(
        ml_dtypes.bfloat16
    )

    # dinv^2 per core/window/row
    d2pad = np.ones((NCORES, nwpad), dtype=np.float32)
    d2 = (dinv * dinv).astype(np.float32)
    for d in range(NCORES):
        d2pad[d, :per] = d2[d * per : (d + 1) * per]
    dinv2 = np.transpose(d2pad.reshape(NCORES, nwin, 128), (0, 2, 1)).copy()

    iota = np.zeros((128, cfg.iota_len), dtype=np.float32)
    for gt in cfg.gts:
        o = cfg.iota_off[gt]
        vals = np.repeat(np.arange(128, dtype=np.float32), gt)
        iota[:, o : o + gt * 128] = vals[None, :]
    ident = np.eye(128, dtype=np.float32)

    common = {
        "w1": np.asarray(W1, np.float32).astype(ml_dtypes.bfloat16),
        "w2": np.asarray(W2, np.float32).astype(ml_dtypes.bfloat16),
        "woutT": np.ascontiguousarray(np.asarray(Wout, np.float32).T).astype(
            ml_dtypes.bfloat16
        ),
        "b1": np.asarray(b1, np.float32).reshape(128, 1).copy(),
        "b2": np.asarray(b2, np.float32).reshape(128, 1).copy(),
        "boutbc": np.broadcast_to(
            np.asarray(bout, np.float32)[None, :], (128, NPRED)
        ).copy(),
        "iotaT": iota.astype(ml_dtypes.bfloat16),
        "ident": ident.astype(ml_dtypes.bfloat16),
    }
    in_maps = []
    for d in range(NCORES):
        m = dict(common)
        m["colv"] = colv_t[d]
        m["wv"] = wv_t[d]
        m["idx"] = idx_wrapped[d]
        m["xTloc"] = xTloc[d]
        m["dinv2"] = dinv2[d]
        m["counts"] = counts2[d : d + 1]
        in_maps.append(m)
    return in_maps


_CACHE = {}


def run(cfg, x, edge_index, edge_weight, W1, b1, W2, b2, Wout, bout):
    in_maps = prep_inputs(cfg, x, edge_index, edge_weight, W1, b1, W2, b2, Wout, bout)
    key = (cfg.n, cfg.cap)
    if key not in _CACHE:
        _CACHE[key] = build_nc(cfg)
    nc = _CACHE[key]
    res = run_bass_kernel_spmd(nc, in_maps, list(range(NCORES)))
    outs = []
    for d in range(NCORES):
        ot = res.results[d]["outT2"]  # [nwpad, 16]
        outs.append(ot[: cfg.per, :])
    return np.ascontiguousarray(np.concatenate(outs, axis=0), dtype=np.float32)


def kernel(x, edge_index, edge_weight, W1, b1, W2, b2, Wout, bout):
    cfg = Cfg(100000, 640)
    return run(cfg, x, edge_index, edge_weight, W1, b1, W2, b2, Wout, bout)


if __name__ == "__main__":
    rng = np.random.default_rng(0)
    n, e = 4096, 65536
    x = rng.standard_normal((n, 128)).astype(np.float32)
    ei = rng.integers(0, n, (2, e)).astype(np.int64)
    ew = rng.random(e).astype(np.float32)
    W1 = (rng.standard_normal((128, 128)) / np.sqrt(128)).astype(np.float32)
    W2 = (rng.standard_normal((128, 128)) / np.sqrt(128)).astype(np.float32)
    Wout = (rng.standard_normal((16, 128)) / np.sqrt(128)).astype(np.float32)
    b1 = rng.standard_normal(128).astype(np.float32) * 0.1
    b2 = rng.standard_normal(128).astype(np.float32) * 0.1
    bout = rng.standard_normal(16).astype(np.float32) * 0.1

    def gcn(xx, W, b):
        deg = np.bincount(ei[1], weights=ew, minlength=n) + 1.0
        dinv = 1.0 / np.sqrt(deg)
        xw = xx @ W
        msg = xw[ei[0]] * (dinv[ei[0]] * ew * dinv[ei[1]])[:, None]
        out = np.zeros_like(xw)
        np.add.at(out, ei[1], msg)
        out += xw * (dinv**2)[:, None]
        return np.maximum(out + b, 0.0)

    h = gcn(x, W1, b1)
    h = gcn(h, W2, b2)
    ref = h @ Wout.T + bout

    cfg = Cfg(n, 768)
    got = run(cfg, x, ei, ew, W1, b1, W2, b2, Wout, bout)
    err = np.abs(got - ref).max() / (np.abs(ref).max() + 1e-9)
    l2 = np.linalg.norm(got - ref) / np.linalg.norm(ref)
    print(f"SMOKE: max rel err {err:.3e}   l2 rel {l2:.3e}")
